# revision 1
# baseline (speedup 1.0000x reference)
"""CIN (Compressed Interaction Network) kernel for Trainium2, 8 NeuronCores.

Problem: x (2048, 39, 16) f32; 3 CIN layers with W_i (200, 39, prev):
    z[b,o,d] = sum_{f,g} W[o,f,g] * x0[b,f,d] * h[b,g,d] + bias[o]
    h' = relu(z);  output = sum_d concat([h1,h2,h3], ch) -> (2048, 600)

Strategy (data-parallel over batch, 8 cores, 256 batch rows each):
  Per core, columns n = (b_local, d), N = 256*16 = 4096, in 16 n-tiles of 256
  (two 128-column windows each).  Matmuls run in the z^T orientation:
  out psum [128 n, 200 o] accumulates over the contraction (f, g) --
  streaming all 200 output channels in one pass instead of two 128/72
  splits.  lhsT (stationary) = V slices [g, n-window]; rhs (moving) =
  weight slices [g, 200].  V_f = h (.) bcast(x0[f]):
    f 0..7   built in fp8 directly on the GPSIMD/Pool engine
    f 8..15  built fp16 on Vector, converted to fp8 by the Scalar engine
    f 16..38 built fp16 on Vector (widened, 4-8 f's per instruction)
  f 0..15 are consumed by fp8e4m3 DoubleRow matmuls (two f's contracted
  per instruction at 0.5 cycles/row).  All weights are pre-scaled by 64
  so fp8 stays in e4m3's normal range; the relu epilogue on the Scalar
  engine rescales by 1/64.  Bias enters as a K=1 ones-row matmul.
  h^T [n, 200] is transposed back to [g, n] tiles by the PE (identity
  transpose) for the next layer's V build; d-sums run on the Vector
  engine.  Tiles are emitted pairwise-interleaved so the PE always has
  an independent tile's matmuls (keeps the p-state clock ramped).
"""
import numpy as np

import concourse.bacc as bacc
import concourse.mybir as mybir
import concourse.tile as tile
from concourse.bass_utils import run_bass_kernel_spmd

B, F0, D = 2048, 39, 16
C = 200                      # cross size per layer
NCORES = 8
BC = B // NCORES             # 256 batch rows per core
N = BC * D                   # 4096 columns per core
NT = 256                     # n-tile width
T = N // NT                  # 16 n-tiles
BT = NT // D                 # 16 batch rows per n-tile
NW = NT // 128               # 2 matmul windows per tile
KF0 = (F0 * F0 + 127) // 128  # 12 flat L0 K-chunks (zero-padded)
NF8 = 16                     # f's 0..NF8-1 in fp8 DoubleRow (pairs)
NFP = 8                      # of those, f's 0..NFP-1 built on Pool
NP8 = NF8 // 2               # 8 pairs
NF16 = F0 - NF8              # 23 f's in fp16 matmuls
NV16 = F0 - NFP              # 31 f's built in fp16 on Vector (f 8..38)
GA, GB = 128, C - 128        # g-chunks (h partition split 128 + 72)
SCALE = 64.0                 # weight pre-scale (power of 2)
F16 = mybir.dt.float16
F8 = mybir.dt.float8e4
F32 = mybir.dt.float32


def _build_nc():
    nc = bacc.Bacc(None, target_bir_lowering=False)
    mult = mybir.AluOpType.mult
    relu = mybir.ActivationFunctionType.Relu

    x0_d = nc.dram_tensor("x0", [F0, N], F16, kind="ExternalInput")
    v0_d = nc.dram_tensor("v0", [KF0 * 128, N], F16, kind="ExternalInput")
    w0_d = nc.dram_tensor("w0", [128, KF0 * C], F16, kind="ExternalInput")
    w1a_d = nc.dram_tensor("w1a", [GA, NF16 * C], F16, kind="ExternalInput")
    w1b_d = nc.dram_tensor("w1b", [GB, NF16 * C], F16, kind="ExternalInput")
    w2a_d = nc.dram_tensor("w2a", [GA, NF16 * C], F16, kind="ExternalInput")
    w2b_d = nc.dram_tensor("w2b", [GB, NF16 * C], F16, kind="ExternalInput")
    w18a_d = nc.dram_tensor("w18a", [GA, NF8 * C], F8, kind="ExternalInput")
    w18b_d = nc.dram_tensor("w18b", [GB, NF8 * C], F8, kind="ExternalInput")
    w28a_d = nc.dram_tensor("w28a", [GA, NF8 * C], F8, kind="ExternalInput")
    w28b_d = nc.dram_tensor("w28b", [GB, NF8 * C], F8, kind="ExternalInput")
    brow_d = nc.dram_tensor("brow", [1, 3 * C], F16, kind="ExternalInput")
    ones_d = nc.dram_tensor("ones1", [1, 128], F16, kind="ExternalInput")
    id_d = nc.dram_tensor("ident", [128, 128], F16, kind="ExternalInput")
    out_d = nc.dram_tensor("out3", [3, C, BC], F32, kind="ExternalOutput")

    with tile.TileContext(nc) as tc:
        with (
            tc.tile_pool(name="wp", bufs=1) as wp,
            tc.tile_pool(name="bc", bufs=2) as bcp,
            tc.tile_pool(name="hp", bufs=2) as hp,
            tc.tile_pool(name="ht", bufs=4) as htp,
            tc.tile_pool(name="va", bufs=2) as vap,
            tc.tile_pool(name="ps", bufs=4, space="PSUM") as ps,
            tc.tile_pool(name="pt", bufs=2, space="PSUM") as pt,
        ):
            # --- static state -------------------------------------------------
            w0 = wp.tile([128, KF0 * C], F16)
            nc.sync.dma_start(out=w0[:], in_=w0_d[:])
            brow = wp.tile([1, 3 * C], F16)
            nc.sync.dma_start(out=brow[:], in_=brow_d[:])
            ones1 = wp.tile([1, 128], F16)
            nc.sync.dma_start(out=ones1[:], in_=ones_d[:])
            ident = wp.tile([128, 128], F16)
            nc.sync.dma_start(out=ident[:], in_=id_d[:])
            outs = []
            for l in range(3):
                oa = wp.tile([GA, BC], F32, tag=f"o{l}a")
                ob = wp.tile([GB, BC], F32, tag=f"o{l}b")
                outs.append((oa, ob))

            def emit_v0(t):
                v0t = bcp.tile([128, KF0 * NT], F16, tag="v0t")
                src = (v0_d[:].rearrange("(c p) n -> p c n", p=128)
                       [:, :, t * NT:(t + 1) * NT])
                for c0 in range(0, KF0, 6):
                    c1 = min(c0 + 6, KF0)
                    nc.sync.dma_start(
                        out=v0t[:, c0 * NT:c1 * NT]
                        .rearrange("p (c n) -> p c n", n=NT),
                        in_=src[:, c0:c1, :])
                return v0t

            def emit_xb(t, fchunk=13):
                xb = bcp.tile([128, F0 * NT], F16, tag="xb")
                for f0 in range(0, F0, fchunk):
                    f1 = min(f0 + fchunk, F0)
                    src = (x0_d[f0:f1, t * NT:(t + 1) * NT]
                           .unsqueeze(0).broadcast_to((128, f1 - f0, NT)))
                    nc.sync.dma_start(
                        out=xb[:, f0 * NT:f1 * NT]
                        .rearrange("p (f n) -> p f n", n=NT), in_=src)
                return xb

            def emit_build(xb, ha, hb):
                # V tiles for one (tile, layer):
                #   va8/vb8 [*, NF8*NT] f8: f 0..NFP-1 Pool-direct,
                #     f NFP..NF8-1 Act-converted from va
                #   va/vb [*, NV16*NT] f16: f NFP..38 on Vector
                va = vap.tile([GA, NV16 * NT], F16, tag="va")
                vb = vap.tile([GB, NV16 * NT], F16, tag="vb")
                va8 = vap.tile([GA, NF8 * NT], F8, tag="va8")
                vb8 = vap.tile([GB, NF8 * NT], F8, tag="vb8")
                fs = slice(0, NFP * NT)
                nc.gpsimd.tensor_tensor(
                    out=va8[:, fs].rearrange("p (f n) -> p f n", n=NT),
                    in0=ha[:].unsqueeze(1).broadcast_to((GA, NFP, NT)),
                    in1=xb[0:GA, fs].rearrange("p (f n) -> p f n", n=NT),
                    op=mult)
                nc.gpsimd.tensor_tensor(
                    out=vb8[:, fs].rearrange("p (f n) -> p f n", n=NT),
                    in0=hb[:].unsqueeze(1).broadcast_to((GB, NFP, NT)),
                    in1=xb[0:GB, fs].rearrange("p (f n) -> p f n", n=NT),
                    op=mult)
                for j0 in range(0, NV16, 8):
                    j1 = min(j0 + 8, NV16)
                    w = j1 - j0
                    fs = slice((NFP + j0) * NT, (NFP + j1) * NT)
                    nc.vector.tensor_tensor(
                        out=va[:, j0 * NT:j1 * NT]
                        .rearrange("p (f n) -> p f n", n=NT),
                        in0=ha[:].unsqueeze(1).broadcast_to((GA, w, NT)),
                        in1=xb[0:GA, fs].rearrange("p (f n) -> p f n", n=NT),
                        op=mult)
                    nc.vector.tensor_tensor(
                        out=vb[:, j0 * NT:j1 * NT]
                        .rearrange("p (f n) -> p f n", n=NT),
                        in0=hb[:].unsqueeze(1).broadcast_to((GB, w, NT)),
                        in1=xb[0:GB, fs].rearrange("p (f n) -> p f n", n=NT),
                        op=mult)
                # Act converts va positions 0..NF8-NFP-1 (f NFP..NF8-1)
                cs = slice(0, (NF8 - NFP) * NT)
                c8 = slice(NFP * NT, NF8 * NT)
                nc.scalar.copy(out=va8[:, c8], in_=va[:, cs])
                nc.scalar.copy(out=vb8[:, c8], in_=vb[:, cs])
                return va, vb, va8, vb8

            def emit_l0_win(v0t, w, l):
                # z^T psum [128 n, 200] over 12 flat K-chunks + bias row
                pz = ps.tile([128, C], F32, tag="pz")
                v3 = v0t[:].rearrange("p (c n) -> p c n", n=NT)
                for c in range(KF0):
                    nc.tensor.matmul(pz[:], v3[:, c, w * 128:(w + 1) * 128],
                                     w0[:, c * C:(c + 1) * C],
                                     start=(c == 0), stop=False)
                nc.tensor.matmul(pz[:], ones1[:],
                                 brow[:, l * C:(l + 1) * C],
                                 start=False, stop=True)
                return pz

            def emit_l12_win(vs, wa, wb, w8a, w8b, w, l):
                va, vb, va8, vb8 = vs
                pz = ps.tile([128, C], F32, tag="pz")
                ws = slice(w * 128, (w + 1) * 128)
                v3a = va[:].rearrange("p (f n) -> p f n", n=NT)
                v3b = vb[:].rearrange("p (f n) -> p f n", n=NT)
                for j in range(NF16):
                    # fp16 f = NF8 + j lives at va position (NF8 - NFP) + j
                    p = (NF8 - NFP) + j
                    nc.tensor.matmul(pz[:], v3a[:, p, ws],
                                     wa[:, j * C:(j + 1) * C],
                                     start=(j == 0), stop=False)
                    nc.tensor.matmul(pz[:], v3b[:, p, ws],
                                     wb[:, j * C:(j + 1) * C],
                                     start=False, stop=False)
                p3a = va8[:].rearrange("p (f n) -> p f n", n=NT)
                p3b = vb8[:].rearrange("p (f n) -> p f n", n=NT)
                w4a = w8a[:].rearrange("p (j t o) -> p j t o", t=2, o=C)
                w4b = w8b[:].rearrange("p (j t o) -> p j t o", t=2, o=C)
                for j in range(NP8):
                    nc.tensor.matmul(pz[:], p3a[:, 2 * j:2 * j + 2, ws],
                                     w4a[:, j], start=False, stop=False,
                                     perf_mode=mybir.MatmulPerfMode.DoubleRow)
                    nc.tensor.matmul(pz[:], p3b[:, 2 * j:2 * j + 2, ws],
                                     w4b[:, j], start=False, stop=False,
                                     perf_mode=mybir.MatmulPerfMode.DoubleRow)
                nc.tensor.matmul(pz[:], ones1[:],
                                 brow[:, l * C:(l + 1) * C],
                                 start=False, stop=True)
                return pz

            def emit_epi(pz, w, ha, hb):
                # relu (+1/64 rescale) -> h^T [128, 200]; transpose to h tiles
                hT = htp.tile([128, C], F16, tag="hT")
                nc.scalar.activation(hT[:], pz[:], relu, scale=1.0 / SCALE)
                pa = pt.tile([128, 128], F16, tag="pta")
                pb = pt.tile([GB, 128], F16, tag="ptb")
                nc.tensor.transpose(pa[:], hT[:, 0:GA], ident[:])
                nc.tensor.transpose(pb[:], hT[:, GA:C], ident[:])
                ws = slice(w * 128, (w + 1) * 128)
                nc.scalar.copy(out=ha[:, ws], in_=pa[:])
                nc.scalar.copy(out=hb[:, ws], in_=pb[:])

            def emit_reduce(t, l, ha, hb):
                oa, ob = outs[l]
                bs = slice(t * BT, (t + 1) * BT)
                nc.vector.tensor_reduce(
                    out=oa[:, bs], in_=ha[:].rearrange("p (b d) -> p b d", d=D),
                    axis=mybir.AxisListType.X, op=mybir.AluOpType.add)
                nc.vector.tensor_reduce(
                    out=ob[:, bs], in_=hb[:].rearrange("p (b d) -> p b d", d=D),
                    axis=mybir.AxisListType.X, op=mybir.AluOpType.add)

            # --- pipeline: pairwise-interleaved n-tiles ----------------------
            v00, v01 = emit_v0(0), emit_v0(1)
            xb0, xb1 = emit_xb(0), emit_xb(1)
            w1a = wp.tile([GA, NF16 * C], F16)
            nc.sync.dma_start(out=w1a[:], in_=w1a_d[:])
            w1b = wp.tile([GB, NF16 * C], F16)
            nc.sync.dma_start(out=w1b[:], in_=w1b_d[:])
            w18a = wp.tile([GA, NF8 * C], F8)
            nc.sync.dma_start(out=w18a[:], in_=w18a_d[:])
            w18b = wp.tile([GB, NF8 * C], F8)
            nc.sync.dma_start(out=w18b[:], in_=w18b_d[:])
            w2a = wp.tile([GA, NF16 * C], F16)
            nc.sync.dma_start(out=w2a[:], in_=w2a_d[:])
            w2b = wp.tile([GB, NF16 * C], F16)
            nc.sync.dma_start(out=w2b[:], in_=w2b_d[:])
            w28a = wp.tile([GA, NF8 * C], F8)
            nc.sync.dma_start(out=w28a[:], in_=w28a_d[:])
            w28b = wp.tile([GB, NF8 * C], F8)
            nc.sync.dma_start(out=w28b[:], in_=w28b_d[:])

            for tp in range(0, T, 2):
                t0, t1 = tp, tp + 1
                hs = []
                for k in range(2):
                    row = []
                    for l in range(3):
                        hta = hp.tile([GA, NT], F16, tag=f"h{l}a{k}",
                                      name=f"h{l}a{k}_{tp}")
                        htb = hp.tile([GB, NT], F16, tag=f"h{l}b{k}",
                                      name=f"h{l}b{k}_{tp}")
                        row.append((hta, htb))
                    hs.append(tuple(row))
                v0s, xbs = (v00, v01), (xb0, xb1)
                # L0 both tiles, window-interleaved
                for w in range(NW):
                    for k in range(2):
                        pz = emit_l0_win(v0s[k], w, 0)
                        emit_epi(pz, w, *hs[k][0])
                # build V1 + L1 both tiles
                vss = []
                for k, t in ((0, t0), (1, t1)):
                    emit_reduce(t, 0, *hs[k][0])
                    vss.append(emit_build(xbs[k], *hs[k][0]))
                for w in range(NW):
                    for k in range(2):
                        pz = emit_l12_win(vss[k], w1a, w1b, w18a, w18b, w, 1)
                        emit_epi(pz, w, *hs[k][1])
                # build V2 + L2 both tiles
                vss = []
                for k, t in ((0, t0), (1, t1)):
                    emit_reduce(t, 1, *hs[k][1])
                    vss.append(emit_build(xbs[k], *hs[k][1]))
                for w in range(NW):
                    for k in range(2):
                        pz = emit_l12_win(vss[k], w2a, w2b, w28a, w28b, w, 2)
                        emit_epi(pz, w, *hs[k][2])
                for k, t in ((0, t0), (1, t1)):
                    emit_reduce(t, 2, *hs[k][2])
                if tp + 2 < T:
                    v00, v01 = emit_v0(tp + 2), emit_v0(tp + 3)
                    xb0, xb1 = emit_xb(tp + 2), emit_xb(tp + 3)

            for l in range(3):
                oa, ob = outs[l]
                nc.sync.dma_start(out=out_d[l, 0:GA, :], in_=oa[:])
                nc.sync.dma_start(out=out_d[l, GA:C, :], in_=ob[:])

    nc.compile()
    return nc


_NC_CACHE = None


def _get_nc():
    global _NC_CACHE
    if _NC_CACHE is None:
        _NC_CACHE = _build_nc()
    return _NC_CACHE


def _q8(x):
    import ml_dtypes
    return np.asarray(x, np.float32).astype(ml_dtypes.float8_e4m3fn)


def _prep_weights(W0, b0, W1, b1, W2, b2):
    # L0 rhs layout: w0[p, c*C+o] = SCALE*W0[o, f, g], flat k=128c+p=f*39+g
    W0 = np.asarray(W0, np.float32)
    w0f = np.zeros((KF0 * 128, C), np.float32)
    w0f[0:F0 * F0] = W0.reshape(C, F0 * F0).T * SCALE
    w0 = np.ascontiguousarray(
        w0f.reshape(KF0, 128, C).transpose(1, 0, 2).reshape(128, KF0 * C)
    ).astype(np.float16)

    def lay(W):
        # Wt[g, f, o] = SCALE*W[o, f, g]
        Wt = np.asarray(W, np.float32).transpose(2, 1, 0) * SCALE
        wa = np.ascontiguousarray(Wt[0:GA, NF8:].reshape(GA, NF16 * C)
                                  ).astype(np.float16)
        wb = np.ascontiguousarray(Wt[GA:C, NF8:].reshape(GB, NF16 * C)
                                  ).astype(np.float16)
        w8a = _q8(np.ascontiguousarray(Wt[0:GA, 0:NF8].reshape(GA, NF8 * C)))
        w8b = _q8(np.ascontiguousarray(Wt[GA:C, 0:NF8].reshape(GB, NF8 * C)))
        return wa, wb, w8a, w8b

    w1a, w1b, w18a, w18b = lay(W1)
    w2a, w2b, w28a, w28b = lay(W2)
    brow = np.zeros((1, 3 * C), np.float16)
    for l, b in enumerate((b0, b1, b2)):
        brow[0, l * C:(l + 1) * C] = (np.asarray(b, np.float32) * SCALE
                                      ).astype(np.float16)
    return {
        "w0": w0, "w1a": w1a, "w1b": w1b, "w2a": w2a, "w2b": w2b,
        "w18a": w18a, "w18b": w18b, "w28a": w28a, "w28b": w28b,
        "brow": brow,
        "ones1": np.ones((1, 128), np.float16),
        "ident": np.eye(128, dtype=np.float16),
    }


def kernel(x, W0, b0, W1, b1, W2, b2):
    x = np.asarray(x)
    assert x.shape == (B, F0, D), x.shape
    nc = _get_nc()
    shared = _prep_weights(W0, b0, W1, b1, W2, b2)

    in_maps = []
    for c in range(NCORES):
        xc = x[c * BC:(c + 1) * BC]                      # [256, 39, 16]
        x0c = np.ascontiguousarray(
            xc.transpose(1, 0, 2).reshape(F0, N)).astype(np.float16)
        x0f32 = x0c.astype(np.float32)
        v0 = np.zeros((KF0 * 128, N), np.float16)
        v0[0:F0 * F0] = (x0f32[:, None, :] * x0f32[None, :, :]
                         ).reshape(F0 * F0, N).astype(np.float16)
        in_maps.append({"x0": x0c, "v0": v0, **shared})

    res = run_bass_kernel_spmd(nc, in_maps, list(range(NCORES)))

    out = np.empty((B, 3 * C), dtype=np.float32)
    for c in range(NCORES):
        o3 = res.results[c]["out3"]                      # [3, 200, 256]
        out[c * BC:(c + 1) * BC] = o3.transpose(2, 0, 1).reshape(BC, 3 * C)
    return out



# revision 5
# speedup vs baseline: 1.1603x; 1.1603x over previous
"""CIN (Compressed Interaction Network) kernel for Trainium2, 8 NeuronCores.

Problem: x (2048, 39, 16) f32; 3 CIN layers with W_i (200, 39, prev):
    z[b,o,d] = sum_{f,g} W[o,f,g] * x0[b,f,d] * h[b,g,d] + bias[o]
    h' = relu(z);  output = sum_d concat([h1,h2,h3], ch) -> (2048, 600)

Strategy (data-parallel over batch, 8 cores, 256 batch rows each):
  Per core, columns n = (b_local, d), N = 256*16 = 4096, in 16 n-tiles of 256
  (two 128-column windows each).  Matmuls run in the z^T orientation:
  psum [128 n, 200 o] accumulates over the contraction (f, g); lhsT
  (stationary) = V slices [g, n-window]; rhs (moving) = weight slices
  [g, 200].  V_f = h (.) bcast(x0[f]) is split across engines by f-slot:
    slots 0..PF-1         fp8 direct on Pool (scalar_tensor_tensor,
                          which costs 0.60-efficiency vs 0.42 for mult)
    slots PF..PF+YF-1     fp16 on Vector (tensor_tensor, 2x mode),
                          converted to fp8 by the Scalar engine
    slots PF+YF..38       fp16 on Vector, consumed by fp16 matmuls
  fp8 slots feed fp8e4m3 DoubleRow matmuls (two f's per instruction at
  0.5 cycles/row).  Weights are pre-scaled by 64 so fp8 stays in e4m3's
  normal range; the relu epilogue on the Scalar engine rescales by 1/64.
  Bias enters as a K=1 ones-row matmul.  L0 uses the f<=g symmetry of
  x0*x0: 780 rows (7 K-chunks) with folded weights W0+W0^T.
  h^T [n, 200] is transposed by the PE into a PSUM bank; the Vector
  engine copies it to SBUF h tiles [g, n] for the next layer's V build.
  The d-sums run on the PE as tiny K=128 matmuls against a 0/1 selector
  [128, 8], accumulated in a PSUM bank that is DMA'd straight to DRAM
  every 4 tiles.  Tiles are emitted pairwise-interleaved so the PE
  always has an independent tile's matmuls (keeps the clock ramped).
"""
import numpy as np

import concourse.bacc as bacc
import concourse.mybir as mybir
import concourse.tile as tile
from concourse.bass_utils import run_bass_kernel_spmd

B, F0, D = 2048, 39, 16
C = 200                      # cross size per layer
NCORES = 8
BC = B // NCORES             # 256 batch rows per core
N = BC * D                   # 4096 columns per core
NT = 256                     # n-tile width
T = N // NT                  # 16 n-tiles
BT = NT // D                 # 16 batch rows per n-tile
NW = NT // 128               # 2 matmul windows per tile
K0 = 7                       # L0 symmetric K-chunks (780 rows padded to 896)
NPAIR = F0 * (F0 + 1) // 2   # 780
PF = 11                      # f-slots built fp8 directly on Pool
YF = 15                      # f-slots built fp16 on Vector, Act-converted
XF = F0 - PF - YF            # 13 f-slots kept fp16 end to end
NF8 = PF + YF                # 26 fp8 slots
NP8 = NF8 // 2               # 13 DoubleRow pairs
NV16 = YF + XF               # 28 f-slots built on Vector (fp16)
GA, GB = 128, C - 128        # g-split (h partition split 128 + 72)
SCALE = 64.0                 # weight pre-scale (power of 2)
GRP = 4                      # tiles per output-psum flush group
F16 = mybir.dt.float16
F8 = mybir.dt.float8e4
F32 = mybir.dt.float32


def _build_nc():
    nc = bacc.Bacc(None, target_bir_lowering=False)
    mult = mybir.AluOpType.mult
    relu = mybir.ActivationFunctionType.Relu

    x0_d = nc.dram_tensor("x0", [F0, N], F16, kind="ExternalInput")
    v0_d = nc.dram_tensor("v0", [K0 * 128, N], F16, kind="ExternalInput")
    w0_d = nc.dram_tensor("w0", [128, K0 * C], F16, kind="ExternalInput")
    w1a_d = nc.dram_tensor("w1a", [GA, XF * C], F16, kind="ExternalInput")
    w1b_d = nc.dram_tensor("w1b", [GB, XF * C], F16, kind="ExternalInput")
    w2a_d = nc.dram_tensor("w2a", [GA, XF * C], F16, kind="ExternalInput")
    w2b_d = nc.dram_tensor("w2b", [GB, XF * C], F16, kind="ExternalInput")
    w18a_d = nc.dram_tensor("w18a", [GA, NF8 * C], F8, kind="ExternalInput")
    w18b_d = nc.dram_tensor("w18b", [GB, NF8 * C], F8, kind="ExternalInput")
    w28a_d = nc.dram_tensor("w28a", [GA, NF8 * C], F8, kind="ExternalInput")
    w28b_d = nc.dram_tensor("w28b", [GB, NF8 * C], F8, kind="ExternalInput")
    brow_d = nc.dram_tensor("brow", [1, 3 * C], F16, kind="ExternalInput")
    ones_d = nc.dram_tensor("ones1", [1, 128], F16, kind="ExternalInput")
    id_d = nc.dram_tensor("ident", [128, 128], F16, kind="ExternalInput")
    smat_d = nc.dram_tensor("smat", [128, BT // NW], F16, kind="ExternalInput")
    outa_d = nc.dram_tensor("outa", [GA, 3 * N // D], F32, kind="ExternalOutput")
    outb_d = nc.dram_tensor("outb", [GB, 3 * N // D], F32, kind="ExternalOutput")

    with tile.TileContext(nc) as tc:
        with (
            tc.tile_pool(name="wp", bufs=1) as wp,
            tc.tile_pool(name="bc", bufs=2) as bcp,
            tc.tile_pool(name="hs", bufs=4) as hsp,
            tc.tile_pool(name="ht", bufs=4) as htp,
            tc.tile_pool(name="va", bufs=2) as vap,
            tc.tile_pool(name="ps", bufs=3, space="PSUM") as ps,
            tc.tile_pool(name="pt", bufs=3, space="PSUM") as pt,
            tc.tile_pool(name="op", bufs=2, space="PSUM") as opp,
        ):
            # --- static state -------------------------------------------------
            w0 = wp.tile([128, K0 * C], F16)
            nc.sync.dma_start(out=w0[:], in_=w0_d[:])
            brow = wp.tile([1, 3 * C], F16)
            nc.sync.dma_start(out=brow[:], in_=brow_d[:])
            ones1 = wp.tile([1, 128], F16)
            nc.sync.dma_start(out=ones1[:], in_=ones_d[:])
            ident = wp.tile([128, 128], F16)
            nc.sync.dma_start(out=ident[:], in_=id_d[:])
            smat = wp.tile([128, BT // NW], F16)
            nc.sync.dma_start(out=smat[:], in_=smat_d[:])
            outa_s = wp.tile([GA, 3 * N // D], F32)
            outb_s = wp.tile([GB, 3 * N // D], F32)

            def emit_v0(t):
                v0t = bcp.tile([128, K0 * NT], F16, tag="v0t")
                src = (v0_d[:].rearrange("(c p) n -> p c n", p=128)
                       [:, :, t * NT:(t + 1) * NT])
                nc.sync.dma_start(
                    out=v0t[:].rearrange("p (c n) -> p c n", n=NT), in_=src)
                return v0t

            def emit_xb(t, fchunk=13):
                xb = bcp.tile([128, F0 * NT], F16, tag="xb")
                for f0 in range(0, F0, fchunk):
                    f1 = min(f0 + fchunk, F0)
                    src = (x0_d[f0:f1, t * NT:(t + 1) * NT]
                           .unsqueeze(0).broadcast_to((128, f1 - f0, NT)))
                    nc.sync.dma_start(
                        out=xb[:, f0 * NT:f1 * NT]
                        .rearrange("p (f n) -> p f n", n=NT), in_=src)
                return xb

            def emit_build(xb, ha, hb):
                # V tiles for one (tile, layer):
                #   va8/vb8 [*, NF8*NT] f8: slots 0..PF-1 Pool-direct,
                #     slots PF..NF8-1 Act-converted from va cols 0..YF-1
                #   va/vb [*, NV16*NT] f16: col j <-> f-slot PF+j
                va = vap.tile([GA, NV16 * NT], F16, tag="va")
                vb = vap.tile([GB, NV16 * NT], F16, tag="vb")
                va8 = vap.tile([GA, NF8 * NT], F8, tag="va8")
                vb8 = vap.tile([GB, NF8 * NT], F8, tag="vb8")
                fs = slice(0, PF * NT)
                nc.gpsimd.scalar_tensor_tensor(
                    out=va8[:, fs].rearrange("p (f n) -> p f n", n=NT),
                    in0=ha[:].unsqueeze(1).broadcast_to((GA, PF, NT)),
                    scalar=1.0,
                    in1=xb[0:GA, fs].rearrange("p (f n) -> p f n", n=NT),
                    op0=mult, op1=mult)
                nc.gpsimd.scalar_tensor_tensor(
                    out=vb8[:, fs].rearrange("p (f n) -> p f n", n=NT),
                    in0=hb[:].unsqueeze(1).broadcast_to((GB, PF, NT)),
                    scalar=1.0,
                    in1=xb[0:GB, fs].rearrange("p (f n) -> p f n", n=NT),
                    op0=mult, op1=mult)
                # Vector builds: chunk 1 = conv slots (feeds Act), chunk 2 = fp16
                for j0, j1 in ((0, YF), (YF, NV16)):
                    w = j1 - j0
                    fs = slice((PF + j0) * NT, (PF + j1) * NT)
                    nc.vector.tensor_tensor(
                        out=va[:, j0 * NT:j1 * NT]
                        .rearrange("p (f n) -> p f n", n=NT),
                        in0=ha[:].unsqueeze(1).broadcast_to((GA, w, NT)),
                        in1=xb[0:GA, fs].rearrange("p (f n) -> p f n", n=NT),
                        op=mult)
                    nc.vector.tensor_tensor(
                        out=vb[:, j0 * NT:j1 * NT]
                        .rearrange("p (f n) -> p f n", n=NT),
                        in0=hb[:].unsqueeze(1).broadcast_to((GB, w, NT)),
                        in1=xb[0:GB, fs].rearrange("p (f n) -> p f n", n=NT),
                        op=mult)
                    if j0 == 0:
                        cs = slice(0, YF * NT)
                        c8 = slice(PF * NT, NF8 * NT)
                        nc.scalar.copy(out=va8[:, c8], in_=va[:, cs])
                        nc.scalar.copy(out=vb8[:, c8], in_=vb[:, cs])
                return va, vb, va8, vb8

            def emit_l0_win(v0t, w, l):
                pz = ps.tile([128, C], F32, tag="pz")
                nc.tensor.matmul(pz[:], ones1[:],
                                 brow[:, l * C:(l + 1) * C],
                                 start=True, stop=False)
                v3 = v0t[:].rearrange("p (c n) -> p c n", n=NT)
                for c in range(K0):
                    nc.tensor.matmul(pz[:], v3[:, c, w * 128:(w + 1) * 128],
                                     w0[:, c * C:(c + 1) * C],
                                     start=False, stop=(c == K0 - 1))
                return pz

            def emit_l12_win(vs, wa, wb, w8a, w8b, w, l):
                va, vb, va8, vb8 = vs
                pz = ps.tile([128, C], F32, tag="pz")
                ws = slice(w * 128, (w + 1) * 128)
                nc.tensor.matmul(pz[:], ones1[:],
                                 brow[:, l * C:(l + 1) * C],
                                 start=True, stop=False)
                v3a = va[:].rearrange("p (f n) -> p f n", n=NT)
                v3b = vb[:].rearrange("p (f n) -> p f n", n=NT)
                for j in range(XF):
                    # fp16 f-slot PF+YF+j lives at va col YF+j
                    p = YF + j
                    nc.tensor.matmul(pz[:], v3a[:, p, ws],
                                     wa[:, j * C:(j + 1) * C],
                                     start=False, stop=False)
                    nc.tensor.matmul(pz[:], v3b[:, p, ws],
                                     wb[:, j * C:(j + 1) * C],
                                     start=False, stop=False)
                p3a = va8[:].rearrange("p (f n) -> p f n", n=NT)
                p3b = vb8[:].rearrange("p (f n) -> p f n", n=NT)
                w4a = w8a[:].rearrange("p (j t o) -> p j t o", t=2, o=C)
                w4b = w8b[:].rearrange("p (j t o) -> p j t o", t=2, o=C)
                for j in range(NP8):
                    nc.tensor.matmul(pz[:], p3a[:, 2 * j:2 * j + 2, ws],
                                     w4a[:, j], start=False, stop=False,
                                     perf_mode=mybir.MatmulPerfMode.DoubleRow)
                    nc.tensor.matmul(pz[:], p3b[:, 2 * j:2 * j + 2, ws],
                                     w4b[:, j], start=False, stop=(j == NP8 - 1),
                                     perf_mode=mybir.MatmulPerfMode.DoubleRow)
                return pz

            def emit_epi(pz, w, l, t, pout, hps=None, ha=None, hb=None):
                # relu (+1/64 rescale) -> h^T [128, 200]; d-sum on the PE;
                # for l<2: transpose into a PSUM bank, copy to SBUF h tiles
                hT = htp.tile([128, C], F16, tag="hT")
                nc.scalar.activation(hT[:], pz[:], relu, scale=1.0 / SCALE)
                if l < 2:
                    ws = slice(w * 128, (w + 1) * 128)
                    ws2 = slice(NT + w * 128, NT + (w + 1) * 128)
                    nc.tensor.transpose(hps[:, ws], hT[:, 0:GA], ident[:])
                    nc.tensor.transpose(hps[0:GB, ws2], hT[:, GA:C], ident[:])
                    nc.vector.tensor_copy(out=ha[:, ws], in_=hps[:, ws])
                    nc.vector.tensor_copy(out=hb[:, ws], in_=hps[0:GB, ws2])
                off = l * 64 + (t % GRP) * BT + w * (BT // NW)
                nc.tensor.matmul(pout[:, off:off + BT // NW],
                                 hT[:, 0:GA], smat[:], start=True, stop=True)
                nc.tensor.matmul(pout[0:GB, 192 + off:192 + off + BT // NW],
                                 hT[:, GA:C], smat[:], start=True, stop=True)

            # --- weights ------------------------------------------------------
            v00, v01 = emit_v0(0), emit_v0(1)
            xb0, xb1 = emit_xb(0), emit_xb(1)
            w1a = wp.tile([GA, XF * C], F16)
            nc.sync.dma_start(out=w1a[:], in_=w1a_d[:])
            w1b = wp.tile([GB, XF * C], F16)
            nc.sync.dma_start(out=w1b[:], in_=w1b_d[:])
            w18a = wp.tile([GA, NF8 * C], F8)
            nc.sync.dma_start(out=w18a[:], in_=w18a_d[:])
            w18b = wp.tile([GB, NF8 * C], F8)
            nc.sync.dma_start(out=w18b[:], in_=w18b_d[:])
            w2a = wp.tile([GA, XF * C], F16)
            nc.sync.dma_start(out=w2a[:], in_=w2a_d[:])
            w2b = wp.tile([GB, XF * C], F16)
            nc.sync.dma_start(out=w2b[:], in_=w2b_d[:])
            w28a = wp.tile([GA, NF8 * C], F8)
            nc.sync.dma_start(out=w28a[:], in_=w28a_d[:])
            w28b = wp.tile([GB, NF8 * C], F8)
            nc.sync.dma_start(out=w28b[:], in_=w28b_d[:])

            # --- pipeline: pairwise-interleaved n-tiles ----------------------
            pout = None
            for tp in range(0, T, 2):
                t0, t1 = tp, tp + 1
                if tp % GRP == 0:
                    pout = opp.tile([128, 2 * 192], F32, tag="pout")
                v0s, xbs = (v00, v01), (xb0, xb1)
                hts = []
                for k in range(2):
                    row = []
                    for l in range(2):
                        hps = pt.tile([128, 2 * NT], F16, tag="hps",
                                      name=f"hps{l}_{tp}_{k}")
                        ha = hsp.tile([GA, NT], F16, tag=f"h{l}a{k}",
                                      name=f"h{l}a{k}_{tp}")
                        hb = hsp.tile([GB, NT], F16, tag=f"h{l}b{k}",
                                      name=f"h{l}b{k}_{tp}")
                        row.append((hps, ha, hb))
                    hts.append(row)
                # L0 both tiles, window-interleaved
                for w in range(NW):
                    for k, t in ((0, t0), (1, t1)):
                        pz = emit_l0_win(v0s[k], w, 0)
                        emit_epi(pz, w, 0, t, pout, *hts[k][0])
                # build V1 + L1 both tiles
                vss = [emit_build(xbs[k], hts[k][0][1], hts[k][0][2])
                       for k in range(2)]
                for w in range(NW):
                    for k, t in ((0, t0), (1, t1)):
                        pz = emit_l12_win(vss[k], w1a, w1b, w18a, w18b, w, 1)
                        emit_epi(pz, w, 1, t, pout, *hts[k][1])
                # build V2 + L2 both tiles
                vss = [emit_build(xbs[k], hts[k][1][1], hts[k][1][2])
                       for k in range(2)]
                for w in range(NW):
                    for k, t in ((0, t0), (1, t1)):
                        pz = emit_l12_win(vss[k], w2a, w2b, w28a, w28b, w, 2)
                        emit_epi(pz, w, 2, t, pout)
                if tp % GRP == GRP - 2:
                    g = tp // GRP
                    nc.scalar.copy(out=outa_s[:, g * 192:(g + 1) * 192],
                                   in_=pout[:, 0:192])
                    nc.scalar.copy(out=outb_s[:, g * 192:(g + 1) * 192],
                                   in_=pout[0:GB, 192:384])
                if tp + 2 < T:
                    v00, v01 = emit_v0(tp + 2), emit_v0(tp + 3)
                    xb0, xb1 = emit_xb(tp + 2), emit_xb(tp + 3)

            nc.sync.dma_start(out=outa_d[:], in_=outa_s[:])
            nc.sync.dma_start(out=outb_d[:], in_=outb_s[:])

    nc.compile()
    return nc


_NC_CACHE = None


def _get_nc():
    global _NC_CACHE
    if _NC_CACHE is None:
        _NC_CACHE = _build_nc()
    return _NC_CACHE


def _q8(x):
    import ml_dtypes
    return np.asarray(x, np.float32).astype(ml_dtypes.float8_e4m3fn)


def _prep_weights(W0, b0, W1, b1, W2, b2):
    # L0: symmetric fold.  W0eff[o, (f,g)] = W0[o,f,g]+W0[o,g,f] (f<g),
    # W0[o,f,f] on the diagonal; pairs in triu order, padded 780 -> 896.
    W0 = np.asarray(W0, np.float32)
    iu0, iu1 = np.triu_indices(F0)
    Wsym = W0 + W0.transpose(0, 2, 1)
    dd = np.arange(F0)
    Wsym[:, dd, dd] = W0[:, dd, dd]
    w0f = np.zeros((K0 * 128, C), np.float32)
    w0f[0:NPAIR] = Wsym[:, iu0, iu1].T * SCALE
    w0 = np.ascontiguousarray(
        w0f.reshape(K0, 128, C).transpose(1, 0, 2).reshape(128, K0 * C)
    ).astype(np.float16)

    def lay(W):
        # Wt[g, f, o] = SCALE*W[o, f, g]; slot order = f order
        Wt = np.asarray(W, np.float32).transpose(2, 1, 0) * SCALE
        wa = np.ascontiguousarray(Wt[0:GA, NF8:].reshape(GA, XF * C)
                                  ).astype(np.float16)
        wb = np.ascontiguousarray(Wt[GA:C, NF8:].reshape(GB, XF * C)
                                  ).astype(np.float16)
        w8a = _q8(np.ascontiguousarray(Wt[0:GA, 0:NF8].reshape(GA, NF8 * C)))
        w8b = _q8(np.ascontiguousarray(Wt[GA:C, 0:NF8].reshape(GB, NF8 * C)))
        return wa, wb, w8a, w8b

    w1a, w1b, w18a, w18b = lay(W1)
    w2a, w2b, w28a, w28b = lay(W2)
    brow = np.zeros((1, 3 * C), np.float16)
    for l, b in enumerate((b0, b1, b2)):
        brow[0, l * C:(l + 1) * C] = (np.asarray(b, np.float32) * SCALE
                                      ).astype(np.float16)
    smat = np.zeros((128, BT // NW), np.float16)
    smat[np.arange(128), np.arange(128) // D] = 1.0
    return {
        "w0": w0, "w1a": w1a, "w1b": w1b, "w2a": w2a, "w2b": w2b,
        "w18a": w18a, "w18b": w18b, "w28a": w28a, "w28b": w28b,
        "brow": brow,
        "ones1": np.ones((1, 128), np.float16),
        "ident": np.eye(128, dtype=np.float16),
        "smat": smat,
    }


def kernel(x, W0, b0, W1, b1, W2, b2):
    x = np.asarray(x)
    assert x.shape == (B, F0, D), x.shape
    nc = _get_nc()
    shared = _prep_weights(W0, b0, W1, b1, W2, b2)
    iu0, iu1 = np.triu_indices(F0)

    in_maps = []
    for c in range(NCORES):
        xc = x[c * BC:(c + 1) * BC]                      # [256, 39, 16]
        x0c = np.ascontiguousarray(
            xc.transpose(1, 0, 2).reshape(F0, N)).astype(np.float16)
        x0f32 = x0c.astype(np.float32)
        v0 = np.zeros((K0 * 128, N), np.float16)
        v0[0:NPAIR] = (x0f32[iu0] * x0f32[iu1]).astype(np.float16)
        in_maps.append({"x0": x0c, "v0": v0, **shared})

    res = run_bass_kernel_spmd(nc, in_maps, list(range(NCORES)))

    out = np.empty((B, 3 * C), dtype=np.float32)
    for c in range(NCORES):
        # outa cols: g*192 + l*64 + tl*16 + b16, tiles t = g*4+tl
        oa = res.results[c]["outa"]                      # [128, 768]
        ob = res.results[c]["outb"]                      # [72, 768]
        oa = oa.reshape(GA, 4, 3, GRP, BT).transpose(1, 3, 4, 2, 0)
        ob = ob.reshape(GB, 4, 3, GRP, BT).transpose(1, 3, 4, 2, 0)
        oc = np.concatenate(
            [oa.reshape(BC, 3, GA), ob.reshape(BC, 3, GB)], axis=2)
        out[c * BC:(c + 1) * BC] = oc.reshape(BC, 3 * C)
    return out


# revision 6
# speedup vs baseline: 1.2910x; 1.1126x over previous
"""CIN (Compressed Interaction Network) kernel for Trainium2, 8 NeuronCores.

Problem: x (2048, 39, 16) f32; 3 CIN layers with W_i (200, 39, prev):
    z[b,o,d] = sum_{f,g} W[o,f,g] * x0[b,f,d] * h[b,g,d] + bias[o]
    h' = relu(z);  output = sum_d concat([h1,h2,h3], ch) -> (2048, 600)

Strategy (data-parallel over batch, 8 cores, 256 batch rows each):
  Per core, columns n = (b_local, d), N = 256*16 = 4096, in 16 n-tiles of 256
  (two 128-column windows each).  Matmuls run in the z^T orientation:
  psum [128 n, 200 o] accumulates over the contraction (f, g); lhsT
  (stationary) = V slices [g, n-window]; rhs (moving) = weight slices
  [g, 200].  V_f = h (.) bcast(x0[f]) is split across engines by f-slot:
    slots 0..PF-1         fp8 direct on Pool (scalar_tensor_tensor,
                          which costs 0.60-efficiency vs 0.42 for mult)
    slots PF..PF+YF-1     fp16 on Vector (tensor_tensor, 2x mode),
                          converted to fp8 by the Scalar engine
    slots PF+YF..38       fp16 on Vector, consumed by fp16 matmuls
  fp8 slots feed fp8e4m3 DoubleRow matmuls (two f's per instruction at
  0.5 cycles/row).  Weights are pre-scaled by 64 so fp8 stays in e4m3's
  normal range; the relu epilogue on the Scalar engine rescales by 1/64.
  Bias enters as a K=1 ones-row matmul.  L0 uses the f<=g symmetry of
  x0*x0: 780 rows (7 K-chunks) with folded weights W0+W0^T.
  h^T [n, 200] is transposed by the PE into a PSUM bank; the Vector
  engine copies it to SBUF h tiles [g, n] for the next layer's V build.
  The d-sums run on the PE as tiny K=128 matmuls against a 0/1 selector
  [128, 8], accumulated in a PSUM bank that is DMA'd straight to DRAM
  every 4 tiles.  Tiles are emitted pairwise-interleaved so the PE
  always has an independent tile's matmuls (keeps the clock ramped).
"""
import numpy as np

import concourse.bacc as bacc
import concourse.mybir as mybir
import concourse.tile as tile
from concourse.bass_utils import run_bass_kernel_spmd

B, F0, D = 2048, 39, 16
C = 200                      # cross size per layer
NCORES = 8
BC = B // NCORES             # 256 batch rows per core
N = BC * D                   # 4096 columns per core
NT = 256                     # n-tile width
T = N // NT                  # 16 n-tiles
BT = NT // D                 # 16 batch rows per n-tile
NW = NT // 128               # 2 matmul windows per tile
K0 = 7                       # L0 symmetric K-chunks (780 rows padded to 896)
NPAIR = F0 * (F0 + 1) // 2   # 780
PF = 11                      # f-slots built fp8 directly on Pool
YF = 15                      # f-slots built fp16 on Vector, Act-converted
XF = F0 - PF - YF            # 13 f-slots kept fp16 end to end
NF8 = PF + YF                # 26 fp8 slots
NP8 = NF8 // 2               # 13 DoubleRow pairs
NV16 = YF + XF               # 28 f-slots built on Vector (fp16)
GA, GB = 128, C - 128        # g-split (h partition split 128 + 72)
SCALE = 64.0                 # weight pre-scale (power of 2)
GRP = 4                      # tiles per output-psum flush group
F16 = mybir.dt.float16
F8 = mybir.dt.float8e4
F32 = mybir.dt.float32


def _build_nc():
    nc = bacc.Bacc(None, target_bir_lowering=False)
    mult = mybir.AluOpType.mult
    relu = mybir.ActivationFunctionType.Relu

    x0_d = nc.dram_tensor("x0", [F0, N], F16, kind="ExternalInput")
    v0_d = nc.dram_tensor("v0", [K0 * 128, N], F16, kind="ExternalInput")
    w0_d = nc.dram_tensor("w0", [128, K0 * C], F16, kind="ExternalInput")
    w1a_d = nc.dram_tensor("w1a", [GA, XF * C], F16, kind="ExternalInput")
    w1b_d = nc.dram_tensor("w1b", [GB, XF * C], F16, kind="ExternalInput")
    w2a_d = nc.dram_tensor("w2a", [GA, XF * C], F16, kind="ExternalInput")
    w2b_d = nc.dram_tensor("w2b", [GB, XF * C], F16, kind="ExternalInput")
    w18a_d = nc.dram_tensor("w18a", [GA, NF8 * C], F8, kind="ExternalInput")
    w18b_d = nc.dram_tensor("w18b", [GB, NF8 * C], F8, kind="ExternalInput")
    w28a_d = nc.dram_tensor("w28a", [GA, NF8 * C], F8, kind="ExternalInput")
    w28b_d = nc.dram_tensor("w28b", [GB, NF8 * C], F8, kind="ExternalInput")
    brow_d = nc.dram_tensor("brow", [1, 3 * C], F16, kind="ExternalInput")
    ones_d = nc.dram_tensor("ones1", [1, 128], F16, kind="ExternalInput")
    id_d = nc.dram_tensor("ident", [128, 128], F16, kind="ExternalInput")
    smat_d = nc.dram_tensor("smat", [128, BT // NW], F16, kind="ExternalInput")
    outa_d = nc.dram_tensor("outa", [GA, 3 * N // D], F32, kind="ExternalOutput")
    outb_d = nc.dram_tensor("outb", [GB, 3 * N // D], F32, kind="ExternalOutput")

    with tile.TileContext(nc) as tc:
        with (
            tc.tile_pool(name="wp", bufs=1) as wp,
            tc.tile_pool(name="bc", bufs=2) as bcp,
            tc.tile_pool(name="hs", bufs=4) as hsp,
            tc.tile_pool(name="ht", bufs=4) as htp,
            tc.tile_pool(name="va", bufs=2) as vap,
            tc.tile_pool(name="ps", bufs=3, space="PSUM") as ps,
            tc.tile_pool(name="pt", bufs=3, space="PSUM") as pt,
            tc.tile_pool(name="op", bufs=2, space="PSUM") as opp,
        ):
            # --- static state -------------------------------------------------
            w0 = wp.tile([128, K0 * C], F16)
            nc.sync.dma_start(out=w0[:], in_=w0_d[:])
            brow = wp.tile([1, 3 * C], F16)
            nc.sync.dma_start(out=brow[:], in_=brow_d[:])
            ones1 = wp.tile([1, 128], F16)
            nc.sync.dma_start(out=ones1[:], in_=ones_d[:])
            ident = wp.tile([128, 128], F16)
            nc.sync.dma_start(out=ident[:], in_=id_d[:])
            smat = wp.tile([128, BT // NW], F16)
            nc.sync.dma_start(out=smat[:], in_=smat_d[:])
            outa_s = wp.tile([GA, 3 * N // D], F32)
            outb_s = wp.tile([GB, 3 * N // D], F32)

            def emit_v0(t):
                v0t = bcp.tile([128, K0 * NT], F16, tag="v0t")
                src = (v0_d[:].rearrange("(c p) n -> p c n", p=128)
                       [:, :, t * NT:(t + 1) * NT])
                nc.sync.dma_start(
                    out=v0t[:].rearrange("p (c n) -> p c n", n=NT), in_=src)
                return v0t

            def emit_xb(t, fchunk=13):
                xb = bcp.tile([128, F0 * NT], F16, tag="xb")
                for f0 in range(0, F0, fchunk):
                    f1 = min(f0 + fchunk, F0)
                    src = (x0_d[f0:f1, t * NT:(t + 1) * NT]
                           .unsqueeze(0).broadcast_to((128, f1 - f0, NT)))
                    nc.sync.dma_start(
                        out=xb[:, f0 * NT:f1 * NT]
                        .rearrange("p (f n) -> p f n", n=NT), in_=src)
                return xb

            def emit_build(xb, ha, hb):
                # V tiles for one (tile, layer):
                #   va8/vb8 [*, NF8*NT] f8: slots 0..PF-1 Pool-direct,
                #     slots PF..NF8-1 Act-converted from va cols 0..YF-1
                #   va/vb [*, NV16*NT] f16: col j <-> f-slot PF+j
                va = vap.tile([GA, NV16 * NT], F16, tag="va")
                vb = vap.tile([GB, NV16 * NT], F16, tag="vb")
                va8 = vap.tile([GA, NF8 * NT], F8, tag="va8")
                vb8 = vap.tile([GB, NF8 * NT], F8, tag="vb8")
                fs = slice(0, PF * NT)
                nc.gpsimd.scalar_tensor_tensor(
                    out=va8[:, fs].rearrange("p (f n) -> p f n", n=NT),
                    in0=ha[:].unsqueeze(1).broadcast_to((GA, PF, NT)),
                    scalar=1.0,
                    in1=xb[0:GA, fs].rearrange("p (f n) -> p f n", n=NT),
                    op0=mult, op1=mult)
                nc.gpsimd.scalar_tensor_tensor(
                    out=vb8[:, fs].rearrange("p (f n) -> p f n", n=NT),
                    in0=hb[:].unsqueeze(1).broadcast_to((GB, PF, NT)),
                    scalar=1.0,
                    in1=xb[0:GB, fs].rearrange("p (f n) -> p f n", n=NT),
                    op0=mult, op1=mult)
                # Vector builds: chunk 1 = conv slots (feeds Act), chunk 2 = fp16
                for j0, j1 in ((0, YF), (YF, NV16)):
                    w = j1 - j0
                    fs = slice((PF + j0) * NT, (PF + j1) * NT)
                    nc.vector.tensor_tensor(
                        out=va[:, j0 * NT:j1 * NT]
                        .rearrange("p (f n) -> p f n", n=NT),
                        in0=ha[:].unsqueeze(1).broadcast_to((GA, w, NT)),
                        in1=xb[0:GA, fs].rearrange("p (f n) -> p f n", n=NT),
                        op=mult)
                    nc.vector.tensor_tensor(
                        out=vb[:, j0 * NT:j1 * NT]
                        .rearrange("p (f n) -> p f n", n=NT),
                        in0=hb[:].unsqueeze(1).broadcast_to((GB, w, NT)),
                        in1=xb[0:GB, fs].rearrange("p (f n) -> p f n", n=NT),
                        op=mult)
                    if j0 == 0:
                        cs = slice(0, YF * NT)
                        c8 = slice(PF * NT, NF8 * NT)
                        nc.scalar.copy(out=va8[:, c8], in_=va[:, cs])
                        nc.scalar.copy(out=vb8[:, c8], in_=vb[:, cs])
                return va, vb, va8, vb8

            def emit_l0_win(v0t, w, l):
                pz = ps.tile([128, C], F32, tag="pz")
                nc.tensor.matmul(pz[:], ones1[:],
                                 brow[:, l * C:(l + 1) * C],
                                 start=True, stop=False)
                v3 = v0t[:].rearrange("p (c n) -> p c n", n=NT)
                for c in range(K0):
                    nc.tensor.matmul(pz[:], v3[:, c, w * 128:(w + 1) * 128],
                                     w0[:, c * C:(c + 1) * C],
                                     start=False, stop=(c == K0 - 1))
                return pz

            def emit_l12_win(vs, wa, wb, w8a, w8b, w, l):
                va, vb, va8, vb8 = vs
                pz = ps.tile([128, C], F32, tag="pz")
                ws = slice(w * 128, (w + 1) * 128)
                nc.tensor.matmul(pz[:], ones1[:],
                                 brow[:, l * C:(l + 1) * C],
                                 start=True, stop=False)
                v3a = va[:].rearrange("p (f n) -> p f n", n=NT)
                v3b = vb[:].rearrange("p (f n) -> p f n", n=NT)
                for j in range(XF):
                    # fp16 f-slot PF+YF+j lives at va col YF+j
                    p = YF + j
                    nc.tensor.matmul(pz[:], v3a[:, p, ws],
                                     wa[:, j * C:(j + 1) * C],
                                     start=False, stop=False)
                    nc.tensor.matmul(pz[:], v3b[:, p, ws],
                                     wb[:, j * C:(j + 1) * C],
                                     start=False, stop=False)
                p3a = va8[:].rearrange("p (f n) -> p f n", n=NT)
                p3b = vb8[:].rearrange("p (f n) -> p f n", n=NT)
                w4a = w8a[:].rearrange("p (j t o) -> p j t o", t=2, o=C)
                w4b = w8b[:].rearrange("p (j t o) -> p j t o", t=2, o=C)
                for j in range(NP8):
                    nc.tensor.matmul(pz[:], p3a[:, 2 * j:2 * j + 2, ws],
                                     w4a[:, j], start=False, stop=False,
                                     perf_mode=mybir.MatmulPerfMode.DoubleRow)
                    nc.tensor.matmul(pz[:], p3b[:, 2 * j:2 * j + 2, ws],
                                     w4b[:, j], start=False, stop=(j == NP8 - 1),
                                     perf_mode=mybir.MatmulPerfMode.DoubleRow)
                return pz

            def emit_epi(pz, w, l, t, pout, hps=None, ha=None, hb=None):
                # relu (+1/64 rescale) -> h^T [128, 200]; d-sum on the PE;
                # for l<2: transpose into a PSUM bank, copy to SBUF h tiles
                hT = htp.tile([128, C], F16, tag="hT")
                nc.scalar.activation(hT[:], pz[:], relu, scale=1.0 / SCALE)
                if l < 2:
                    ws = slice(w * 128, (w + 1) * 128)
                    ws2 = slice(NT + w * 128, NT + (w + 1) * 128)
                    nc.tensor.transpose(hps[:, ws], hT[:, 0:GA], ident[:])
                    nc.tensor.transpose(hps[0:GB, ws2], hT[:, GA:C], ident[:])
                    nc.vector.tensor_copy(out=ha[:, ws], in_=hps[:, ws])
                    nc.vector.tensor_copy(out=hb[:, ws], in_=hps[0:GB, ws2])
                off = l * 64 + (t % GRP) * BT + w * (BT // NW)
                nc.tensor.matmul(pout[:, off:off + BT // NW],
                                 hT[:, 0:GA], smat[:], start=True, stop=True)
                nc.tensor.matmul(pout[0:GB, 192 + off:192 + off + BT // NW],
                                 hT[:, GA:C], smat[:], start=True, stop=True)

            # --- weights ------------------------------------------------------
            v00, v01 = emit_v0(0), emit_v0(1)
            xb0, xb1 = emit_xb(0), emit_xb(1)
            w1a = wp.tile([GA, XF * C], F16)
            nc.sync.dma_start(out=w1a[:], in_=w1a_d[:])
            w1b = wp.tile([GB, XF * C], F16)
            nc.sync.dma_start(out=w1b[:], in_=w1b_d[:])
            w18a = wp.tile([GA, NF8 * C], F8)
            nc.sync.dma_start(out=w18a[:], in_=w18a_d[:])
            w18b = wp.tile([GB, NF8 * C], F8)
            nc.sync.dma_start(out=w18b[:], in_=w18b_d[:])
            w2a = wp.tile([GA, XF * C], F16)
            nc.sync.dma_start(out=w2a[:], in_=w2a_d[:])
            w2b = wp.tile([GB, XF * C], F16)
            nc.sync.dma_start(out=w2b[:], in_=w2b_d[:])
            w28a = wp.tile([GA, NF8 * C], F8)
            nc.sync.dma_start(out=w28a[:], in_=w28a_d[:])
            w28b = wp.tile([GB, NF8 * C], F8)
            nc.sync.dma_start(out=w28b[:], in_=w28b_d[:])

            # --- pipeline: rotated so L0(p+1) overlaps V2(p)/L2(p) -----------
            def alloc_h(tp, l):
                hts = []
                for k in range(2):
                    hps = pt.tile([128, 2 * NT], F16, tag="hps",
                                  name=f"hps{l}_{tp}_{k}")
                    ha = hsp.tile([GA, NT], F16, tag=f"h{l}a{k}",
                                  name=f"h{l}a{k}_{tp}")
                    hb = hsp.tile([GB, NT], F16, tag=f"h{l}b{k}",
                                  name=f"h{l}b{k}_{tp}")
                    hts.append((hps, ha, hb))
                return hts

            def emit_l0_pair(tp, v0s, pout):
                hts = alloc_h(tp, 0)
                for w in range(NW):
                    for k in range(2):
                        pz = emit_l0_win(v0s[k], w, 0)
                        emit_epi(pz, w, 0, tp + k, pout, *hts[k])
                return hts

            # prologue: L0 of pair 0
            pout = opp.tile([128, 2 * 192], F32, tag="pout")
            h0s = emit_l0_pair(0, (v00, v01), pout)
            xbs = (xb0, xb1)

            for tp in range(0, T, 2):
                t0, t1 = tp, tp + 1
                # prefetch pair p+1 inputs early
                if tp + 2 < T:
                    v0n = (emit_v0(tp + 2), emit_v0(tp + 3))
                    xbn = (emit_xb(tp + 2), emit_xb(tp + 3))
                # V1 builds + L1 windows
                h1s = alloc_h(tp, 1)
                vss = [emit_build(xbs[k], h0s[k][1], h0s[k][2])
                       for k in range(2)]
                for w in range(NW):
                    for k, t in ((0, t0), (1, t1)):
                        pz = emit_l12_win(vss[k], w1a, w1b, w18a, w18b, w, 1)
                        emit_epi(pz, w, 1, t, pout, *h1s[k])
                # V2 builds; L0 of pair p+1 runs on the PE meanwhile
                vss = [emit_build(xbs[k], h1s[k][1], h1s[k][2])
                       for k in range(2)]
                pout_cur = pout
                if tp + 2 < T:
                    if (tp + 2) % GRP == 0:
                        pout = opp.tile([128, 2 * 192], F32, tag="pout")
                    h0s = emit_l0_pair(tp + 2, v0n, pout)
                    xbs = xbn
                # L2 windows
                for w in range(NW):
                    for k, t in ((0, t0), (1, t1)):
                        pz = emit_l12_win(vss[k], w2a, w2b, w28a, w28b, w, 2)
                        emit_epi(pz, w, 2, t, pout_cur)
                if tp % GRP == GRP - 2:
                    g = tp // GRP
                    nc.scalar.copy(out=outa_s[:, g * 192:(g + 1) * 192],
                                   in_=pout_cur[:, 0:192])
                    nc.scalar.copy(out=outb_s[:, g * 192:(g + 1) * 192],
                                   in_=pout_cur[0:GB, 192:384])

            nc.sync.dma_start(out=outa_d[:], in_=outa_s[:])
            nc.sync.dma_start(out=outb_d[:], in_=outb_s[:])

    nc.compile()
    return nc


_NC_CACHE = None


def _get_nc():
    global _NC_CACHE
    if _NC_CACHE is None:
        _NC_CACHE = _build_nc()
    return _NC_CACHE


def _q8(x):
    import ml_dtypes
    return np.asarray(x, np.float32).astype(ml_dtypes.float8_e4m3fn)


def _prep_weights(W0, b0, W1, b1, W2, b2):
    # L0: symmetric fold.  W0eff[o, (f,g)] = W0[o,f,g]+W0[o,g,f] (f<g),
    # W0[o,f,f] on the diagonal; pairs in triu order, padded 780 -> 896.
    W0 = np.asarray(W0, np.float32)
    iu0, iu1 = np.triu_indices(F0)
    Wsym = W0 + W0.transpose(0, 2, 1)
    dd = np.arange(F0)
    Wsym[:, dd, dd] = W0[:, dd, dd]
    w0f = np.zeros((K0 * 128, C), np.float32)
    w0f[0:NPAIR] = Wsym[:, iu0, iu1].T * SCALE
    w0 = np.ascontiguousarray(
        w0f.reshape(K0, 128, C).transpose(1, 0, 2).reshape(128, K0 * C)
    ).astype(np.float16)

    def lay(W):
        # Wt[g, f, o] = SCALE*W[o, f, g]; slot order = f order
        Wt = np.asarray(W, np.float32).transpose(2, 1, 0) * SCALE
        wa = np.ascontiguousarray(Wt[0:GA, NF8:].reshape(GA, XF * C)
                                  ).astype(np.float16)
        wb = np.ascontiguousarray(Wt[GA:C, NF8:].reshape(GB, XF * C)
                                  ).astype(np.float16)
        w8a = _q8(np.ascontiguousarray(Wt[0:GA, 0:NF8].reshape(GA, NF8 * C)))
        w8b = _q8(np.ascontiguousarray(Wt[GA:C, 0:NF8].reshape(GB, NF8 * C)))
        return wa, wb, w8a, w8b

    w1a, w1b, w18a, w18b = lay(W1)
    w2a, w2b, w28a, w28b = lay(W2)
    brow = np.zeros((1, 3 * C), np.float16)
    for l, b in enumerate((b0, b1, b2)):
        brow[0, l * C:(l + 1) * C] = (np.asarray(b, np.float32) * SCALE
                                      ).astype(np.float16)
    smat = np.zeros((128, BT // NW), np.float16)
    smat[np.arange(128), np.arange(128) // D] = 1.0
    return {
        "w0": w0, "w1a": w1a, "w1b": w1b, "w2a": w2a, "w2b": w2b,
        "w18a": w18a, "w18b": w18b, "w28a": w28a, "w28b": w28b,
        "brow": brow,
        "ones1": np.ones((1, 128), np.float16),
        "ident": np.eye(128, dtype=np.float16),
        "smat": smat,
    }


def kernel(x, W0, b0, W1, b1, W2, b2):
    x = np.asarray(x)
    assert x.shape == (B, F0, D), x.shape
    nc = _get_nc()
    shared = _prep_weights(W0, b0, W1, b1, W2, b2)
    iu0, iu1 = np.triu_indices(F0)

    in_maps = []
    for c in range(NCORES):
        xc = x[c * BC:(c + 1) * BC]                      # [256, 39, 16]
        x0c = np.ascontiguousarray(
            xc.transpose(1, 0, 2).reshape(F0, N)).astype(np.float16)
        x0f32 = x0c.astype(np.float32)
        v0 = np.zeros((K0 * 128, N), np.float16)
        v0[0:NPAIR] = (x0f32[iu0] * x0f32[iu1]).astype(np.float16)
        in_maps.append({"x0": x0c, "v0": v0, **shared})

    res = run_bass_kernel_spmd(nc, in_maps, list(range(NCORES)))

    out = np.empty((B, 3 * C), dtype=np.float32)
    for c in range(NCORES):
        # outa cols: g*192 + l*64 + tl*16 + b16, tiles t = g*4+tl
        oa = res.results[c]["outa"]                      # [128, 768]
        ob = res.results[c]["outb"]                      # [72, 768]
        oa = oa.reshape(GA, 4, 3, GRP, BT).transpose(1, 3, 4, 2, 0)
        ob = ob.reshape(GB, 4, 3, GRP, BT).transpose(1, 3, 4, 2, 0)
        oc = np.concatenate(
            [oa.reshape(BC, 3, GA), ob.reshape(BC, 3, GB)], axis=2)
        out[c * BC:(c + 1) * BC] = oc.reshape(BC, 3 * C)
    return out
